# revision 1
# baseline (speedup 1.0000x reference)
"""ListMLE-with-tail loss kernel for Trainium2 (Bass/Tile), 8-core data-parallel.

Full-input contract: kernel(output[1024,50000] f32, target[1024] i32,
tails[1024,50] i32, tail_len[1024] i32) -> neg_like[1024] f32.

Sharding: batch rows split 128 per core (one row per SBUF partition).
Per core the kernel streams the [128, 50000] row-slice through the scalar
engine's exp with fused per-chunk row-sum accumulation (accum_out), gathers
the 51 needed scores per row (target + reversed tails) with one indirect
DMA, and computes the tail term with a tensor_tensor_scan cumsum plus a
log-with-bias activation. Host-side preprocessing is limited to index/mask
arithmetic (gather indices, validity mask) — all touches of `output` data
happen on device.
"""

import functools

import numpy as np

import concourse.bass as bass
import concourse.bacc as bacc
import concourse.tile as tile
from concourse import mybir
from concourse.bass_utils import run_bass_kernel_spmd

B = 1024
V = 50000
T = 50
M = 8            # cores
P = B // M       # 128 rows per core = SBUF partitions
C = 6250         # free-dim chunk of the exp-sum stream
NCH = V // C     # 8 chunks
G = T + 1        # gathered scores per row: [target, reversed tails]

F32 = mybir.dt.float32
I32 = mybir.dt.int32


def _build_program() -> bass.Bass:
    nc = bacc.Bacc()
    x = nc.dram_tensor("x", [P, V], F32, kind="ExternalInput")
    gidx = nc.dram_tensor("gidx", [P, G], I32, kind="ExternalInput")
    maskr = nc.dram_tensor("maskr", [P, T], F32, kind="ExternalInput")
    loss = nc.dram_tensor("loss", [P, 1], F32, kind="ExternalOutput")

    with tile.TileContext(nc) as tc:
        with (
            tc.tile_pool(name="inp", bufs=3) as inp,
            tc.tile_pool(name="scratch", bufs=2) as scratch,
            tc.tile_pool(name="small", bufs=1) as small,
        ):
            # Small per-row tensors: gather indices, validity mask.
            gidx_t = small.tile([P, G], I32)
            nc.sync.dma_start(out=gidx_t[:], in_=gidx[:])
            maskr_t = small.tile([P, T], F32)
            nc.sync.dma_start(out=maskr_t[:], in_=maskr[:])

            # sg[p, 0] = x[p, target[p]]; sg[p, 1+t] = x[p, tails[p, T-1-t]]
            # HW indirect DMA consumes one index per partition per op (the
            # [P, G] offset-AP form silently uses only column 0), so gather
            # column-by-column: op k does sg[p, k] = x_flat[gidx[p, k]].
            sg = small.tile([P, G], F32)
            xflat = x[:].rearrange("p (v u) -> (p v) u", u=1)
            for k in range(G):
                nc.gpsimd.indirect_dma_start(
                    out=sg[:, k:k + 1],
                    out_offset=None,
                    in_=xflat,
                    in_offset=bass.IndirectOffsetOnAxis(ap=gidx_t[:, k:k + 1], axis=0),
                )
            # Funnel DMA-produced tiles through one DVE copy each so no
            # downstream instruction needs >1 cross-engine sync wait (the
            # TensorTensor encoding carries a single wait slot).
            maskr2 = small.tile([P, T], F32)
            nc.vector.tensor_copy(out=maskr2[:], in_=maskr_t[:])
            sg2 = small.tile([P, G], F32)
            nc.vector.tensor_copy(out=sg2[:], in_=sg[:])

            # Main stream: total_exp[p] = sum_v exp(x[p, v]), chunked.
            sums = small.tile([P, NCH], F32)
            for i in range(NCH):
                xt = inp.tile([P, C], F32)
                nc.sync.dma_start(out=xt[:], in_=x[:, i * C:(i + 1) * C])
                et = scratch.tile([P, C], F32, tag="exp_scratch")
                nc.scalar.activation(
                    out=et[:],
                    in_=xt[:],
                    func=mybir.ActivationFunctionType.Exp,
                    accum_out=sums[:, i:i + 1],
                )
            total = small.tile([P, 1], F32)
            nc.vector.reduce_sum(out=total[:], in_=sums[:], axis=mybir.AxisListType.X)

            # Tail term, all [P, <=51] ops.
            e_all = small.tile([P, G], F32)
            nc.scalar.activation(
                out=e_all[:], in_=sg[:], func=mybir.ActivationFunctionType.Exp
            )
            es = small.tile([P, T], F32)
            nc.vector.tensor_mul(out=es[:], in0=e_all[:, 1:G], in1=maskr2[:])
            # c[p, t] = cumsum of es along t == reference's cumsum of flipped es.
            c = small.tile([P, T], F32)
            nc.vector.tensor_tensor_scan(
                out=c[:],
                data0=es[:],
                data1=es[:],
                initial=0.0,
                op0=mybir.AluOpType.add,
                op1=mybir.AluOpType.bypass,
            )
            # others = total - exp(target_score) - sum(es); sum(es) = c[:, -1]
            others = small.tile([P, 1], F32)
            nc.vector.tensor_scalar(
                out=others[:],
                in0=total[:],
                scalar1=e_all[:, 0:1],
                scalar2=c[:, T - 1:T],
                op0=mybir.AluOpType.subtract,
                op1=mybir.AluOpType.subtract,
            )
            # lg = log(c + others)
            lg = small.tile([P, T], F32)
            nc.scalar.activation(
                out=lg[:],
                in_=c[:],
                func=mybir.ActivationFunctionType.Ln,
                bias=others[:],
            )
            wl = small.tile([P, T], F32)
            nc.vector.tensor_mul(out=wl[:], in0=lg[:], in1=maskr2[:])
            below = small.tile([P, 1], F32)
            nc.vector.reduce_sum(out=below[:], in_=wl[:], axis=mybir.AxisListType.X)
            sm = small.tile([P, T], F32)
            nc.vector.tensor_mul(out=sm[:], in0=sg2[:, 1:G], in1=maskr2[:])
            above = small.tile([P, 1], F32)
            nc.vector.reduce_sum(out=above[:], in_=sm[:], axis=mybir.AxisListType.X)

            # loss = -(target_score - log(total) + above - below)
            logtot = small.tile([P, 1], F32)
            nc.scalar.activation(
                out=logtot[:], in_=total[:], func=mybir.ActivationFunctionType.Ln
            )
            t1 = small.tile([P, 1], F32)
            nc.vector.tensor_scalar(
                out=t1[:],
                in0=logtot[:],
                scalar1=sg2[:, 0:1],
                scalar2=above[:],
                op0=mybir.AluOpType.subtract,
                op1=mybir.AluOpType.subtract,
            )
            res = small.tile([P, 1], F32)
            nc.vector.tensor_add(out=res[:], in0=t1[:], in1=below[:])
            nc.sync.dma_start(out=loss[:], in_=res[:])
    nc.finalize()  # runs the bacc passes (sync-wait splitting etc.)
    return nc


@functools.cache
def _program() -> bass.Bass:
    return _build_program()


def _prep_core_inputs(output, target, tails, tail_len, core):
    r0 = core * P
    x = np.ascontiguousarray(output[r0:r0 + P]).astype(np.float32, copy=False)
    tgt = target[r0:r0 + P].astype(np.int64)
    tls = tails[r0:r0 + P].astype(np.int64)
    tln = tail_len[r0:r0 + P].astype(np.int64)

    row = np.arange(P, dtype=np.int64)[:, None] * V
    gidx = np.empty((P, G), dtype=np.int32)
    gidx[:, 0] = (row[:, 0] + tgt).astype(np.int32)
    gidx[:, 1:] = (row + tls[:, ::-1]).astype(np.int32)
    # maskr[r, t] = 1 iff reversed-tail position t is valid: (T-1-t) < tail_len[r]
    tpos = np.arange(T - 1, -1, -1, dtype=np.int64)[None, :]
    maskr = (tpos < tln[:, None]).astype(np.float32)
    return {"x": x, "gidx": gidx, "maskr": np.ascontiguousarray(maskr)}


def kernel(output, target, tails, tail_len):
    output = np.asarray(output, dtype=np.float32)
    target = np.asarray(target)
    tails = np.asarray(tails)
    tail_len = np.asarray(tail_len)

    in_maps = [
        _prep_core_inputs(output, target, tails, tail_len, core) for core in range(M)
    ]
    out = run_bass_kernel_spmd(_program(), in_maps, core_ids=list(range(M)))
    global last_result
    last_result = out
    return np.concatenate(
        [r["loss"].reshape(P).astype(np.float32) for r in out.results]
    )


last_result = None



# revision 9
# speedup vs baseline: 1.0647x; 1.0647x over previous
"""ListMLE-with-tail loss kernel for Trainium2 (Bass/Tile), 8-core data-parallel.

Full-input contract: kernel(output[1024,50000] f32, target[1024] i32,
tails[1024,50] i32, tail_len[1024] i32) -> neg_like[1024] f32.

Sharding: batch rows split 128 per core (one row per SBUF partition).

Per core the kernel streams the [128, 50000] row-slice through the scalar
engine's exp (in place) with fused per-chunk row-sum accumulation
(accum_out). The 51 needed scores per row (target + reversed tails) are
picked off the streamed SBUF chunks by one *compressed* gpsimd ap_gather
per chunk: each 16-partition group gathers only the S slots whose column
falls in that chunk (host-packed, zero extra HBM traffic, ~27ns/slot Q7
cost). A final 816-slot ap_gather regroups the accumulated slot buffer
into the k*16+q layout, and a host mask + segmented reduce extracts each
partition's own 51 scores. The tail term (cumsum scan, log-with-bias,
masked reduces) matches the reference math. Host-side preprocessing is
limited to index/mask arithmetic — all touches of `output` data happen on
device.
"""

import functools

import numpy as np

import concourse.bass as bass
import concourse.bacc as bacc
import concourse.tile as tile
from concourse import library_config, mybir
from concourse.bass_utils import run_bass_kernel_spmd

B = 1024
V = 50000
T = 50
M = 8            # cores
P = B // M       # 128 rows per core = SBUF partitions
C = 6250         # free-dim chunk of the exp-sum stream
NCH = V // C     # 8 chunks
G = T + 1        # scores per row: [target, reversed tails]
NI = 16 * G      # final regather slots per 16-partition group
S = 192          # compressed slots per (group, chunk); Binomial(816,1/8)
                 # is 102+-9.5, so 192 is ~9.5 sigma of headroom
SW = S // 16     # wrapped index columns per chunk
IP = 32          # int16 pitch per chunk index block (64B aligned)

F32 = mybir.dt.float32
I16 = mybir.dt.int16


def _build_program(S: int = S) -> bass.Bass:
    SW = S // 16
    nc = bacc.Bacc()
    x = nc.dram_tensor("x", [P, V], F32, kind="ExternalInput")
    # regather idxs [:, 0:G], then chunk c's idxs at [:, G2 + c*IPs :]
    G2 = 64
    IPs = max(IP, -((-S // 16) // 32) * 32)
    gidx = nc.dram_tensor("gidx", [P, G2 + NCH * IPs], I16, kind="ExternalInput")
    m16 = nc.dram_tensor("m16", [P, NI], F32, kind="ExternalInput")
    maskr = nc.dram_tensor("maskr", [P, T], F32, kind="ExternalInput")
    loss = nc.dram_tensor("loss", [P, 1], F32, kind="ExternalOutput")

    with tile.TileContext(nc) as tc:
        with (
            tc.tile_pool(name="inp", bufs=6) as inp,
            tc.tile_pool(name="small", bufs=1) as small,
        ):
            # Start the Q7 ucode swap for ap_gather immediately (~26us,
            # overlaps the leading chunk DMAs).
            nc.gpsimd.load_library(library_config.ap_gather)

            gidx_t = small.tile([P, G2 + NCH * IPs], I16)
            nc.sync.dma_start(out=gidx_t[:], in_=gidx[:])
            # Tail-only inputs ride the ACT engine's HWDGE ring.
            m16_t = small.tile([P, NI], F32)
            nc.scalar.dma_start(out=m16_t[:], in_=m16[:])
            maskr_t = small.tile([P, T], F32)
            nc.scalar.dma_start(out=maskr_t[:], in_=maskr[:])
            maskr2 = small.tile([P, T], F32)
            nc.vector.tensor_copy(out=maskr2[:], in_=maskr_t[:])

            gall = small.tile([P, NCH * S], F32)
            sums = small.tile([P, NCH], F32)

            # Main stream: compressed gather of this chunk's wanted scores,
            # then exp in place with the row-sum accumulator.
            for i in range(NCH):
                xt = inp.tile([P, C], F32)
                nc.sync.dma_start(out=xt[:], in_=x[:, i * C:(i + 1) * C])
                nc.gpsimd.ap_gather(
                    out_ap=gall[:, i * S:(i + 1) * S],
                    in_ap=xt[:],
                    idxs_ap=gidx_t[:, G2 + i * IPs:G2 + i * IPs + SW],
                    channels=P,
                    num_elems=C,
                    d=1,
                    num_idxs=S,
                )
                nc.scalar.activation(
                    out=xt[:],
                    in_=xt[:],
                    func=mybir.ActivationFunctionType.Exp,
                    accum_out=sums[:, i:i + 1],
                )

            # Regroup: rg[p, k*16+q] = gall[p, pos(g, q, k)] (group-shared).
            rg = small.tile([P, NI], F32)
            nc.gpsimd.ap_gather(
                out_ap=rg[:],
                in_ap=gall[:],
                idxs_ap=gidx_t[:, 0:G],
                channels=P,
                num_elems=NCH * S,
                d=1,
                num_idxs=NI,
            )

            total = small.tile([P, 1], F32)
            nc.vector.reduce_sum(out=total[:], in_=sums[:], axis=mybir.AxisListType.X)

            # Extract each partition's own slots: sg[p, k] = rg[p, k*16 + p%16].
            am = small.tile([P, NI], F32)
            nc.vector.tensor_mul(out=am[:], in0=rg[:], in1=m16_t[:])
            sg = small.tile([P, G], F32)
            nc.vector.tensor_reduce(
                out=sg[:],
                in_=am[:].rearrange("p (g s) -> p g s", s=16),
                axis=mybir.AxisListType.X,
                op=mybir.AluOpType.add,
            )

            # Tail term, all [P, <=51] ops.
            e_all = small.tile([P, G], F32)
            nc.scalar.activation(
                out=e_all[:], in_=sg[:], func=mybir.ActivationFunctionType.Exp
            )
            es = small.tile([P, T], F32)
            nc.vector.tensor_mul(out=es[:], in0=e_all[:, 1:G], in1=maskr2[:])
            # c[p, t] = cumsum of es along t == reference's cumsum of flipped es.
            c = small.tile([P, T], F32)
            nc.vector.tensor_tensor_scan(
                out=c[:],
                data0=es[:],
                data1=es[:],
                initial=0.0,
                op0=mybir.AluOpType.add,
                op1=mybir.AluOpType.bypass,
            )
            # others = total - exp(target_score) - sum(es); sum(es) = c[:, -1]
            others = small.tile([P, 1], F32)
            nc.vector.tensor_scalar(
                out=others[:],
                in0=total[:],
                scalar1=e_all[:, 0:1],
                scalar2=c[:, T - 1:T],
                op0=mybir.AluOpType.subtract,
                op1=mybir.AluOpType.subtract,
            )
            # lg = log(c + others)
            lg = small.tile([P, T], F32)
            nc.scalar.activation(
                out=lg[:],
                in_=c[:],
                func=mybir.ActivationFunctionType.Ln,
                bias=others[:],
            )
            wl = small.tile([P, T], F32)
            nc.vector.tensor_mul(out=wl[:], in0=lg[:], in1=maskr2[:])
            below = small.tile([P, 1], F32)
            nc.vector.reduce_sum(out=below[:], in_=wl[:], axis=mybir.AxisListType.X)
            sm = small.tile([P, T], F32)
            nc.vector.tensor_mul(out=sm[:], in0=sg[:, 1:G], in1=maskr2[:])
            above = small.tile([P, 1], F32)
            nc.vector.reduce_sum(out=above[:], in_=sm[:], axis=mybir.AxisListType.X)

            # loss = -(target_score - log(total) + above - below)
            logtot = small.tile([P, 1], F32)
            nc.scalar.activation(
                out=logtot[:], in_=total[:], func=mybir.ActivationFunctionType.Ln
            )
            t1 = small.tile([P, 1], F32)
            nc.vector.tensor_scalar(
                out=t1[:],
                in0=logtot[:],
                scalar1=sg[:, 0:1],
                scalar2=above[:],
                op0=mybir.AluOpType.subtract,
                op1=mybir.AluOpType.subtract,
            )
            res = small.tile([P, 1], F32)
            nc.vector.tensor_add(out=res[:], in0=t1[:], in1=below[:])
            nc.sync.dma_start(out=loss[:], in_=res[:])
    nc.finalize()  # runs the bacc passes (library loads, sync-wait splitting)
    return nc


@functools.cache
def _program(S: int = S) -> bass.Bass:
    return _build_program(S)


# Extraction mask: m16[p, j] = 1 iff slot j belongs to partition p (j%16 == p%16).
_J = np.arange(NI)
_M16 = np.ascontiguousarray(
    ((_J[None, :] % 16) == (np.arange(P)[:, None] % 16)).astype(np.float32)
)
_G2 = 64


def _required_S(target, tails, core):
    r0 = core * P
    tgt = target[r0:r0 + P].astype(np.int64)
    tls = tails[r0:r0 + P].astype(np.int64)
    col = np.empty((P, G), np.int64)
    col[:, 0] = tgt
    col[:, 1:] = tls[:, ::-1]
    cch = col // C
    mx = 0
    for g in range(8):
        sub = cch[g * 16:(g + 1) * 16]
        mx = max(mx, int(np.bincount(sub.reshape(-1), minlength=NCH).max()))
    return mx


def _prep_core_inputs(output, target, tails, tail_len, core, S=S):
    r0 = core * P
    x = np.ascontiguousarray(output[r0:r0 + P]).astype(np.float32, copy=False)
    tgt = target[r0:r0 + P].astype(np.int64)
    tls = tails[r0:r0 + P].astype(np.int64)
    tln = tail_len[r0:r0 + P].astype(np.int64)

    # col[p, 0] = target col; col[p, 1+t] = reversed tail col.
    col = np.empty((P, G), np.int64)
    col[:, 0] = tgt
    col[:, 1:] = tls[:, ::-1]
    cch = col // C                       # owning chunk per (p, k)
    loc = (col % C).astype(np.int64)     # column within the chunk

    IPs = max(IP, -((-S // 16) // 32) * 32)  # per-chunk pitch, 64B aligned
    gidx = np.zeros((P, _G2 + NCH * IPs), np.int16)
    # pos[g*16+q, k] = Gall position (c*S + s) of that (q, k)'s value.
    pos = np.empty((P, G), np.int64)
    for g in range(8):
        qs = np.arange(g * 16, (g + 1) * 16)
        for c in range(NCH):
            qq, kk = np.nonzero(cch[qs] == c)      # group-local (q, k) pairs
            n = len(qq)
            assert n <= S, f"slot budget overflow: {n} > {S}"
            s = np.arange(n)
            pos[qs[0] + qq, kk] = c * S + s
            # chunk-gather idx value at slot s lives wrapped: A[s%16, s//16]
            vals = loc[qs[0] + qq, kk]
            gidx[g * 16 + s % 16, _G2 + c * IPs + s // 16] = vals.astype(np.int16)
    # regather: final slot j = k*16+q reads pos[g*16+q, k]; wrapped layout is
    # exactly [partition g*16+q, column k].
    gidx[:, 0:G] = pos.astype(np.int16)

    # maskr[r, t] = 1 iff reversed-tail position t is valid: (T-1-t) < tail_len[r]
    tpos = np.arange(T - 1, -1, -1, dtype=np.int64)[None, :]
    maskr = (tpos < tln[:, None]).astype(np.float32)
    return {
        "x": x,
        "gidx": gidx,
        "m16": _M16,
        "maskr": np.ascontiguousarray(maskr),
    }


def kernel(output, target, tails, tail_len):
    output = np.asarray(output, dtype=np.float32)
    target = np.asarray(target)
    tails = np.asarray(tails)
    tail_len = np.asarray(tail_len)

    need = max(_required_S(target, tails, core) for core in range(M))
    S_used = max(S, ((need + 15) // 16) * 16)
    in_maps = [
        _prep_core_inputs(output, target, tails, tail_len, core, S_used)
        for core in range(M)
    ]
    out = run_bass_kernel_spmd(_program(S_used), in_maps, core_ids=list(range(M)))
    global last_result
    last_result = out
    return np.concatenate(
        [r["loss"].reshape(P).astype(np.float32) for r in out.results]
    )


last_result = None
